# revision 1
# baseline (speedup 1.0000x reference)
"""Multi-head attention (B=4, S=2048, D=1024, H=16) on 8 TRN2 NeuronCores.

Sharding: core cid handles batch b = cid//2 and head-group hg = cid%2
(8 heads = 512 channels).  Each core computes, for its (b, hg):
  QT = (Wq_hg/8) @ q[b].T + bq/8      [512, 2048]  (channels on partitions)
  KT = Wk_hg @ k[b].T + bk            [512, 2048]
  V  = v[b] @ Wv_hg.T + bv            [2048, 512]  (seq on partitions)
  per head: scoresT = KT_h^T-blocks @ QT_h (contraction d_k=64, two heads
  packed in the 128-partition dim via PE row groups), softmax over the
  partition (S_k) axis computed WITHOUT max-subtraction (scores are O(10);
  exp gets a constant -12 bias that cancels in the normalization) with the
  row-sum obtained for free from a ones-column appended to V,
  attnT accumulated over S_k chunks in PSUM, normalized by the
  PE-broadcast reciprocal of the sums row, written into concatT.
  yT_partial = Wo_hg-rows.T-contraction @ concatT   [1024, 2048]
Host sums the two head-group partials per batch, transposes, adds bo.

All PE operands are fp16 (tf32-class mantissa for these magnitudes);
accumulation is fp32 in PSUM.
"""
import numpy as np
from contextlib import ExitStack, nullcontext

import concourse.bass as bass
import concourse.tile as tile
import concourse.mybir as mybir
import concourse.bass_utils as bass_utils

D_MODEL = 1024
NHEAD = 16
D_K = 64
B = 4
S = 2048
N_CORES = 8
HG = 8            # heads per core
C = HG * D_K      # 512 channels per core
P = 128
EXP_BIAS = -12.0

F16 = mybir.dt.float16
F32 = mybir.dt.float32


def _split_waits(nc, max_waits=1):
    """Cayman CTRL/LW instruction structs carry a single sync-wait slot and
    this walrus rejects instructions with more; move excess SyncWaits onto
    injected same-engine NOPs placed immediately before the instruction."""
    n = 0
    for fn in nc.m.functions:
        for bb in fn.blocks:
            insts = list(bb.instructions)
            out = []
            changed = False
            for inst in insts:
                si = inst.sync_info
                waits = list(si.on_wait) if si is not None and si.on_wait else []
                if len(waits) > max_waits:
                    changed = True
                    extra, keep = waits[:-max_waits], waits[-max_waits:]
                    for w in extra:
                        n += 1
                        nop = mybir.InstNoOp(name=f"wsplit_{n}", ins=[], outs=[])
                        nop.engine = inst.engine
                        nop.sync_info = mybir.SyncInfo(on_wait=[w], on_update=[])
                        out.append(nop)
                    inst.sync_info = mybir.SyncInfo(
                        on_wait=keep,
                        on_update=list(si.on_update) if si.on_update else [],
                    )
                out.append(inst)
            if changed:
                bb.instructions = out
    return n


def build_program(S_=S, reps=1, no_exp=False, no_xdma=False, phases=("v", "qk", "att", "out")):
    NSB = S_ // 512    # 512-wide seq blocks
    NSC = S_ // P      # 128-wide seq chunks
    ND = D_MODEL // P  # model-dim chunks (contraction for projections)
    NCC = C // P       # channel chunks = head pairs

    nc = bass.Bass("TRN2", target_bir_lowering=False, debug=False,
                   num_devices=N_CORES)
    dt_in = F16
    xq = nc.dram_tensor("xqT", [D_MODEL, S_], dt_in, kind="ExternalInput").ap()
    xk = nc.dram_tensor("xkT", [D_MODEL, S_], dt_in, kind="ExternalInput").ap()
    xv = nc.dram_tensor("xvT", [D_MODEL, S_], dt_in, kind="ExternalInput").ap()
    wq = nc.dram_tensor("wqT", [D_MODEL, C], dt_in, kind="ExternalInput").ap()
    wk = nc.dram_tensor("wkT", [D_MODEL, C], dt_in, kind="ExternalInput").ap()
    wv = nc.dram_tensor("wvT", [D_MODEL, C], dt_in, kind="ExternalInput").ap()
    wo = nc.dram_tensor("woT", [C, D_MODEL], dt_in, kind="ExternalInput").ap()
    bq = nc.dram_tensor("bq", [1, C], dt_in, kind="ExternalInput").ap()
    bk = nc.dram_tensor("bk", [1, C], dt_in, kind="ExternalInput").ap()
    bv = nc.dram_tensor("bv", [1, C], dt_in, kind="ExternalInput").ap()
    yTa = nc.dram_tensor("yTa", [D_MODEL, S_], F32, kind="ExternalOutput").ap()
    yTb = nc.dram_tensor("yTb", [D_MODEL, S_], F32, kind="ExternalOutput").ap()

    with tile.TileContext(nc) as tc, ExitStack() as ctx:
        const = ctx.enter_context(tc.tile_pool(name="const", bufs=1))
        big = ctx.enter_context(tc.tile_pool(name="big", bufs=1))
        wpool = ctx.enter_context(tc.tile_pool(name="wp", bufs=1))
        xpool = ctx.enter_context(tc.tile_pool(name="xp", bufs=10))
        epool = ctx.enter_context(tc.tile_pool(name="ep", bufs=6))
        spool = ctx.enter_context(tc.tile_pool(name="sp", bufs=3))
        psum = ctx.enter_context(tc.tile_pool(name="ps", bufs=1, space="PSUM"))

        ones = const.tile([1, 512], F16, tag="ones")
        nc.vector.memset(ones[:], 1.0)
        ebias = const.tile([P, 1], F32, tag="ebias")
        nc.vector.memset(ebias[:], EXP_BIAS)
        bq_sb = const.tile([1, C], F16, tag="bq")
        nc.sync.dma_start(bq_sb[:], bq)
        bk_sb = const.tile([1, C], F16, tag="bk")
        nc.sync.dma_start(bk_sb[:], bk)
        bv_sb = const.tile([1, C], F16, tag="bv")
        nc.sync.dma_start(bv_sb[:], bv)

        QT = big.tile([P, NCC, S_], F16, tag="QT")
        KT = big.tile([P, NCC, S_], F16, tag="KT")
        V = big.tile([P, NSC, HG, 66], F16, tag="V")
        CT = big.tile([P, NCC, S_], F16, tag="CT")
        nc.vector.memset(V[:, :, :, 64:65], 1.0)

        Exp = mybir.ActivationFunctionType.Exp
        mult = mybir.AluOpType.mult
        dummy_e = None
        if no_exp:
            dummy_e = const.tile([P, 1024], F16, tag="dummy_e")
            nc.vector.memset(dummy_e[:], 0.0005)

        # ---- V projection inputs: wv + xvT stay resident ----
        wv_sb = wpool.tile([P, ND, C], F16, tag="wv", name="w_v")
        nc.sync.dma_start(wv_sb[:], wv.rearrange("(c p) m -> p c m", p=P))
        xv_res = big.tile([P, ND, S_], F16, tag="xv")
        for dc in range(ND):
            nc.sync.dma_start(xv_res[:, dc, :], xv[dc * P:(dc + 1) * P, :])

        def proj_v(p):
            """V columns for head pair p only (N=128): one yield per sc group."""
            cs = slice(p * P, (p + 1) * P)
            for sc in range(NSC):
                pt = psum.tile([P, 512], F32, tag="proj", bufs=2,
                               name=f"pv_{p}_{sc}")
                nc.tensor.matmul(pt[:, 0:P], ones[0:1, 0:P], bv_sb[0:1, cs],
                                 start=True, stop=False)
                for dc in range(ND):
                    nc.tensor.matmul(pt[:, 0:P],
                                     xv_res[:, dc, sc * P:(sc + 1) * P],
                                     wv_sb[:, dc, cs], start=False,
                                     stop=(dc == ND - 1))
                nc.vector.tensor_copy(
                    V[:, sc, 2 * p:2 * p + 2, 0:64],
                    pt[:, 0:P].rearrange("p (h d) -> p h d", h=2))
                yield

        def proj_qk(p):
            """QT/KT chunk p: one yield per (tensor, s-block) group."""
            for name, wd, b_sb, out_t in (("q", wq, bq_sb, QT),
                                          ("k", wk, bk_sb, KT)):
                xd = xq if name == "q" else xk
                for sb_ in range(NSB):
                    xts = []
                    for dc in range(ND):
                        xt = xpool.tile([P, 512], F16, tag="xt",
                                        name=f"x{name}_{p}_{sb_}_{dc}")
                        nc.sync.dma_start(
                            xt[:],
                            xd[dc * P:(dc + 1) * P,
                               sb_ * 512:(sb_ + 1) * 512])
                        xts.append(xt)
                    pt = psum.tile([P, 512], F32, tag="proj", bufs=2,
                                   name=f"p{name}_{p}_{sb_}")
                    nc.tensor.matmul(pt[:], b_sb[0:1, p * P:(p + 1) * P],
                                     ones[0:1, :], start=True, stop=False)
                    for dc in range(ND):
                        nc.tensor.matmul(pt[:],
                                         wd_sb[name][:, dc, p * P:(p + 1) * P],
                                         xts[dc][:], start=False,
                                         stop=(dc == ND - 1))
                    nc.vector.tensor_copy(
                        out_t[:, p, sb_ * 512:(sb_ + 1) * 512], pt[:])
                    yield

        def out_proj(half, yT_d):
            """Output projection for CT chunk pair (2*half, 2*half+1)."""
            for mc in range(ND):
                ms = slice(mc * P, (mc + 1) * P)
                for sb_ in range(NSB):
                    ss = slice(sb_ * 512, (sb_ + 1) * 512)
                    pt = psum.tile([P, 512], F32, tag="proj", bufs=2,
                                   name=f"py_{half}_{mc}_{sb_}")
                    for i, pcc in enumerate((2 * half, 2 * half + 1)):
                        nc.tensor.matmul(pt[:], wo_sb[:, pcc, ms],
                                         CT[:, pcc, ss], start=(i == 0),
                                         stop=(i == 1))
                    st = spool.tile([P, 512], F32, tag="stage",
                                    name=f"st_{half}_{mc}_{sb_}")
                    nc.vector.tensor_copy(st[:], pt[:])
                    nc.sync.dma_start(yT_d[ms, ss], st[:])
                    yield

        # weights for q/k/o stay resident (reused across all pairs)
        wd_sb = {}
        for name, wd in (("q", wq), ("k", wk)):
            t = wpool.tile([P, ND, C], F16, tag=f"w{name}", name=f"w_{name}")
            nc.sync.dma_start(t[:], wd.rearrange("(c p) m -> p c m", p=P))
            wd_sb[name] = t
        wo_sb = wpool.tile([P, NCC, D_MODEL], F16, tag="wo", name="w_o")
        nc.sync.dma_start(wo_sb[:], wo.rearrange("(c p) m -> p c m", p=P))

        # ---------- attention: flattened pipelined slot stream ----------
        # One slot = one S_k chunk for one head pair: both heads' scoresT
        # blocks land in a single [128, 1024] PSUM tile (A in [0:512], B in
        # [512:1024]), one exp instruction covers both, and two attn-V
        # matmuls accumulate into the per-head attnT tiles.  Slots are
        # pipelined with a lag of 3 between exp and its attn consumers so
        # every engine handoff is well off the critical path.
        at_tiles = {}

        def emit_scores(p, sq, k):
            qs = slice(sq * 512, (sq + 1) * 512)
            ks = slice(k * P, (k + 1) * P)
            sAB = psum.tile([P, 1024], F32, tag="sc", bufs=2,
                            name=f"sAB_{p}_{sq}_{k}")
            nc.tensor.matmul(sAB[:, 0:512], KT[0:64, p, ks],
                             QT[0:64, p, qs], start=True, stop=True)
            nc.tensor.matmul(sAB[:, 512:1024], KT[64:128, p, ks],
                             QT[64:128, p, qs], start=True, stop=True)
            if no_exp:
                return dummy_e
            eAB = epool.tile([P, 1024], F16, tag="exp", name=f"e_{p}_{sq}_{k}")
            nc.scalar.activation(eAB[:], sAB[:], Exp, bias=ebias[:])
            return eAB

        def emit_attn(p, sq, k, eAB):
            if k == 0:
                at_tiles[(p, sq)] = (
                    psum.tile([P, 512], F32, tag="attn", bufs=2,
                              name=f"atA_{p}_{sq}"),
                    psum.tile([P, 512], F32, tag="attn", bufs=2,
                              name=f"atB_{p}_{sq}"),
                )
            atA, atB = at_tiles[(p, sq)]
            nc.tensor.matmul(atA[0:65], V[:, k, 2 * p, 0:65], eAB[:, 0:512],
                             start=(k == 0), stop=(k == NSC - 1))
            nc.tensor.matmul(atB[0:65], V[:, k, 2 * p + 1, 0:65],
                             eAB[:, 512:1024], start=(k == 0),
                             stop=(k == NSC - 1))

        def emit_norm(p, sq):
            qs = slice(sq * 512, (sq + 1) * 512)
            atA, atB = at_tiles.pop((p, sq))
            for hh, at in ((0, atA), (1, atB)):
                inv = spool.tile([1, 512], F16, tag="inv",
                                 name=f"inv_{p}_{sq}_{hh}")
                with nc.allow_low_precision(
                        reason="softmax 1/sum in fp16: uniform per-column "
                               "scale, ~3e-4 rel err is within budget"):
                    nc.vector.reciprocal(inv[:], at[64:65, :])
                bi = psum.tile([P, 512], F32, tag="proj", bufs=2,
                               name=f"bi_{p}_{sq}_{hh}")
                nc.tensor.matmul(bi[0:64], ones[0:1, 0:64], inv[0:1, :],
                                 start=True, stop=True)
                bis = spool.tile([64, 512], F16, tag="bis",
                                 name=f"bis_{p}_{sq}_{hh}")
                nc.vector.tensor_copy(bis[:], bi[0:64])
                if hh == 0:
                    nc.vector.tensor_tensor(CT[0:64, p, qs], at[0:64, :],
                                            bis[:], mult)
                else:
                    tmp = spool.tile([64, 512], F16, tag="tmpB",
                                     name=f"tmpB_{p}_{sq}")
                    nc.vector.tensor_tensor(tmp[:], at[0:64, :], bis[:], mult)
                    nc.sync.dma_start(CT[64:128, p, qs], tmp[:])

        LAG = 3

        def emit_all():
            from collections import deque
            filler = deque()

            def step_filler(n=1):
                while n > 0 and filler:
                    try:
                        next(filler[0])
                        n -= 1
                    except StopIteration:
                        filler.popleft()

            # lead-in: pair 0 projections emitted densely
            for g in (proj_v(0), proj_qk(0)):
                for _ in g:
                    pass
            pend = deque()

            def pop_attn():
                it = pend.popleft()
                emit_attn(*it)
                if it[2] == NSC - 1:
                    emit_norm(it[0], it[1])

            fill_credit = 0.0
            for p in range(NCC):
                if p + 1 < NCC:
                    filler.append(proj_v(p + 1))
                    filler.append(proj_qk(p + 1))
                if p == NCC - 1:
                    filler.append(out_proj(0, yTa))
                for sq in range(NSB):
                    for k in range(NSC):
                        eAB = emit_scores(p, sq, k)
                        pend.append((p, sq, k, eAB))
                        if len(pend) > LAG:
                            pop_attn()
                        fill_credit += 0.6
                        if fill_credit >= 1.0:
                            fill_credit -= 1.0
                            step_filler(1)
            while pend:
                pop_attn()
            step_filler(10 ** 9)
            for _ in out_proj(1, yTb):
                pass

        loop_cm = tc.For_i(0, reps, 1) if reps > 1 else nullcontext()
        with loop_cm:
            emit_all()

    _split_waits(nc, max_waits=1)
    return nc


_PROGRAM = None


def _get_program():
    global _PROGRAM
    if _PROGRAM is None:
        _PROGRAM = build_program()
    return _PROGRAM


def _make_in_maps(q, k, v, Wq, bq, Wk, bk, Wv, bv, Wo, bo):
    f16 = np.float16
    xqT = [np.ascontiguousarray(q[b].T, dtype=f16) for b in range(B)]
    xkT = [np.ascontiguousarray(k[b].T, dtype=f16) for b in range(B)]
    xvT = [np.ascontiguousarray(v[b].T, dtype=f16) for b in range(B)]
    WqT = np.ascontiguousarray(Wq.T * 0.125, dtype=f16)
    WkT = np.ascontiguousarray(Wk.T, dtype=f16)
    WvT = np.ascontiguousarray(Wv.T, dtype=f16)
    WoT = np.ascontiguousarray(Wo.T, dtype=f16)
    in_maps = []
    for cid in range(N_CORES):
        b, hg = divmod(cid, 2)
        sl = slice(hg * C, (hg + 1) * C)
        in_maps.append({
            "xqT": xqT[b], "xkT": xkT[b], "xvT": xvT[b],
            "wqT": np.ascontiguousarray(WqT[:, sl]),
            "wkT": np.ascontiguousarray(WkT[:, sl]),
            "wvT": np.ascontiguousarray(WvT[:, sl]),
            "woT": np.ascontiguousarray(WoT[sl, :]),
            "bq": (bq[sl] * 0.125).astype(f16).reshape(1, C),
            "bk": bk[sl].astype(f16).reshape(1, C),
            "bv": bv[sl].astype(f16).reshape(1, C),
        })
    return in_maps


def run(inputs, trace=False, trace_cores=None):
    nc = _get_program()
    in_maps = _make_in_maps(**{k: np.asarray(v) for k, v in inputs.items()})
    res = bass_utils.run_bass_kernel_spmd(
        nc, in_maps, core_ids=list(range(N_CORES)), trace=trace,
        trace_cores=trace_cores)
    bo = np.asarray(inputs["bo"], dtype=np.float64)
    out = np.empty((B, S, D_MODEL), np.float32)
    for b in range(B):
        acc = (res.results[2 * b]["yTa"].astype(np.float64)
               + res.results[2 * b]["yTb"].astype(np.float64)
               + res.results[2 * b + 1]["yTa"].astype(np.float64)
               + res.results[2 * b + 1]["yTb"].astype(np.float64)).T + bo
        out[b] = acc.astype(np.float32)
    return out, res


def kernel(**inputs):
    out, _ = run(inputs, trace=False)
    return out



# revision 7
# speedup vs baseline: 1.3684x; 1.3684x over previous
"""Multi-head attention (B=4, S=2048, D=1024, H=16) on 8 TRN2 NeuronCores.

Sharding: core cid handles batch b = cid//2 and head-group hg = cid%2
(8 heads = 512 channels).  Each core computes, for its (b, hg):
  QT = (Wq_hg/8) @ q[b].T + bq/8      [512, 2048]  (channels on partitions)
  KT = Wk_hg @ k[b].T + bk            [512, 2048]
  V  = v[b] @ Wv_hg.T + bv            [2048, 512]  (seq on partitions)
  per head-pair p: scoresT chunk = KT_h^T-blocks @ QT_h (contraction d_k=64,
  two heads packed in the 128-partition dim via PE row tiles -> concurrent),
  softmax WITHOUT max-subtraction (exp bias -12 cancels in normalization),
  row-sums via a ones-column appended to V (65th matmul output row),
  attnT accumulated over S_k chunks in PSUM.
  Normalization is DEFERRED and BATCHED: raw attnT lands in CT (fp16);
  per (pair, sq-block) the two sum rows are staged, DMA-reshaped to
  [8,128], inverted with one reciprocal_approx_fast, DMA'd back to a
  single-partition row, PE-broadcast into a [128,512] psum tile via a
  col-tiled matmul pair, and applied with ONE tensor_tensor multiply.
  This keeps the PE instruction stream dense (no multi-us reciprocal on
  the critical path -> HAM stays at K=8/8).
  yT_partial halves (fp16) per head-pair-pair; host sums 4 partials/batch.

All PE operands are fp16; accumulation is fp32 in PSUM.
"""
import numpy as np
from collections import deque
from contextlib import ExitStack

import concourse.bass as bass
import concourse.tile as tile
import concourse.mybir as mybir
import concourse.bass_utils as bass_utils

D_MODEL = 1024
NHEAD = 16
D_K = 64
B = 4
S = 2048
N_CORES = 8
HG = 8            # heads per core
C = HG * D_K      # 512 channels per core
P = 128
EXP_BIAS = -12.0

F16 = mybir.dt.float16
F32 = mybir.dt.float32


def _split_waits(nc, max_waits=1):
    """Cayman CTRL/LW instruction structs carry a single sync-wait slot and
    this walrus rejects instructions with more; move excess SyncWaits onto
    injected same-engine NOPs placed immediately before the instruction."""
    n = 0
    for fn in nc.m.functions:
        for bb in fn.blocks:
            insts = list(bb.instructions)
            out = []
            changed = False
            for inst in insts:
                si = inst.sync_info
                waits = list(si.on_wait) if si is not None and si.on_wait else []
                if len(waits) > max_waits:
                    changed = True
                    extra, keep = waits[:-max_waits], waits[-max_waits:]
                    for w in extra:
                        n += 1
                        nop = mybir.InstNoOp(name=f"wsplit_{n}", ins=[], outs=[])
                        nop.engine = inst.engine
                        nop.sync_info = mybir.SyncInfo(on_wait=[w], on_update=[])
                        out.append(nop)
                    inst.sync_info = mybir.SyncInfo(
                        on_wait=keep,
                        on_update=list(si.on_update) if si.on_update else [],
                    )
                out.append(inst)
            if changed:
                bb.instructions = out
    return n


def build_program():
    NSB = S // 512     # 4 seq blocks
    NSC = S // P       # 16 seq chunks
    ND = D_MODEL // P  # 8 model-dim chunks
    NCC = C // P       # 4 head pairs

    nc = bass.Bass("TRN2", target_bir_lowering=False, debug=False,
                   num_devices=N_CORES)
    xq = nc.dram_tensor("xqT", [D_MODEL, S], F16, kind="ExternalInput").ap()
    xk = nc.dram_tensor("xkT", [D_MODEL, S], F16, kind="ExternalInput").ap()
    xv = nc.dram_tensor("xvT", [D_MODEL, S], F16, kind="ExternalInput").ap()
    wq = nc.dram_tensor("wqT", [D_MODEL, C], F16, kind="ExternalInput").ap()
    wk = nc.dram_tensor("wkT", [D_MODEL, C], F16, kind="ExternalInput").ap()
    wv = nc.dram_tensor("wvT", [D_MODEL, C], F16, kind="ExternalInput").ap()
    wo = nc.dram_tensor("woT", [C, D_MODEL], F16, kind="ExternalInput").ap()
    bq = nc.dram_tensor("bq", [1, C], F16, kind="ExternalInput").ap()
    bk = nc.dram_tensor("bk", [1, C], F16, kind="ExternalInput").ap()
    bv = nc.dram_tensor("bv", [1, C], F16, kind="ExternalInput").ap()
    yTa = nc.dram_tensor("yTa", [D_MODEL, S], F16, kind="ExternalOutput").ap()
    yTb = nc.dram_tensor("yTb", [D_MODEL, S], F16, kind="ExternalOutput").ap()

    Exp = mybir.ActivationFunctionType.Exp
    mult = mybir.AluOpType.mult

    with tile.TileContext(nc) as tc, ExitStack() as ctx:
        const = ctx.enter_context(tc.tile_pool(name="const", bufs=1))
        big = ctx.enter_context(tc.tile_pool(name="big", bufs=1))
        wpool = ctx.enter_context(tc.tile_pool(name="wp", bufs=1))
        xpool = ctx.enter_context(tc.tile_pool(name="xp", bufs=1))
        epool = ctx.enter_context(tc.tile_pool(name="ep", bufs=12))
        spool = ctx.enter_context(tc.tile_pool(name="sp", bufs=2))
        psum = ctx.enter_context(tc.tile_pool(name="ps", bufs=1, space="PSUM"))

        # ---- constants / weights; DMA order = priority order ----
        ones = const.tile([1, 512], F16, tag="ones")
        nc.vector.memset(ones[:], 1.0)
        ebias = const.tile([P, 1], F32, tag="ebias")
        nc.vector.memset(ebias[:], EXP_BIAS)
        dwarm = const.tile([P, 512], F16, tag="dwarm")
        nc.vector.memset(dwarm[:], 0.001)
        dexp = const.tile([P, 64], F16, tag="dexp")
        bq_sb = const.tile([1, C], F16, tag="bq")
        nc.sync.dma_start(bq_sb[:], bq)
        bk_sb = const.tile([1, C], F16, tag="bk")
        nc.sync.dma_start(bk_sb[:], bk)
        bv_sb = const.tile([1, C], F16, tag="bv")
        nc.sync.dma_start(bv_sb[:], bv)

        QT = big.tile([P, NCC, S], F16, tag="QT")
        KT = big.tile([P, NCC, S], F16, tag="KT")
        V = big.tile([P, NSC, HG, 66], F16, tag="V")
        CT = big.tile([P, NCC, S], F16, tag="CT")
        nc.vector.memset(V[:, :, :, 64:65], 1.0)

        wq_sb = wpool.tile([P, ND, C], F16, tag="wq", name="w_q")
        nc.sync.dma_start(wq_sb[:], wq.rearrange("(c p) m -> p c m", p=P))
        wk_sb = wpool.tile([P, ND, C], F16, tag="wk", name="w_k")
        nc.sync.dma_start(wk_sb[:], wk.rearrange("(c p) m -> p c m", p=P))

        # ACT table preload + PE HAM warm-up while the first DMAs stream.
        nc.scalar.activation(dexp[:], dwarm[:, 0:64], Exp, bias=ebias[:])
        for i in range(28):
            dpt = psum.tile([P, 512], F32, tag="proj", bufs=2, name=f"dw_{i}")
            nc.tensor.matmul(dpt[:], dwarm[:, 0:P], dwarm[:],
                             start=True, stop=True)

        # ---------------- projection generators ----------------
        def qk_proj(p):
            """QT/KT chunk for pair p; one yield per (tensor, s-block)."""
            for name, wd_sb, xd, b_sb, out_t in (
                    ("q", wq_sb, xq, bq_sb, QT), ("k", wk_sb, xk, bk_sb, KT)):
                for sb in range(NSB):
                    xts = []
                    for dc in range(ND):
                        xt = xpool.tile([P, 512], F16, tag="xt", bufs=10,
                                        name=f"x{name}{p}_{sb}_{dc}")
                        nc.sync.dma_start(
                            xt[:], xd[dc * P:(dc + 1) * P,
                                      sb * 512:(sb + 1) * 512])
                        xts.append(xt)
                    pt = psum.tile([P, 512], F32, tag="proj", bufs=2,
                                   name=f"p{name}{p}_{sb}")
                    nc.tensor.matmul(pt[:], b_sb[0:1, p * P:(p + 1) * P],
                                     ones[0:1, :], start=True, stop=False)
                    for dc in range(ND):
                        nc.tensor.matmul(pt[:], wd_sb[:, dc, p * P:(p + 1) * P],
                                         xts[dc][:], start=False,
                                         stop=(dc == ND - 1))
                    nc.vector.tensor_copy(
                        out_t[:, p, sb * 512:(sb + 1) * 512], pt[:])
                    yield 2000

        wv_sb = wpool.tile([P, ND, C], F16, tag="wv", name="w_v")
        wo_sb = wpool.tile([P, NCC, D_MODEL], F16, tag="wo", name="w_o")
        xv_tiles = {}

        def v_proj():
            """V for ALL 8 heads per seq chunk (N=512 matmuls)."""
            nc.sync.dma_start(wv_sb[:], wv.rearrange("(c p) m -> p c m", p=P))
            nc.sync.dma_start(wo_sb[:], wo.rearrange("(c p) m -> p c m", p=P))
            for sc in range(NSC):
                sb, j = divmod(sc, 4)
                if j == 0:
                    for dc in range(ND):
                        t = xpool.tile([P, 512], F16, tag="xv", bufs=32,
                                       name=f"xv{sb}_{dc}")
                        nc.sync.dma_start(
                            t[:], xv[dc * P:(dc + 1) * P,
                                     sb * 512:(sb + 1) * 512])
                        xv_tiles[(sb, dc)] = t
                pt = psum.tile([P, 512], F32, tag="proj", bufs=2,
                               name=f"pv_{sc}")
                nc.tensor.matmul(pt[:], ones[0:1, 0:P], bv_sb[0:1, :],
                                 start=True, stop=False)
                for dc in range(ND):
                    nc.tensor.matmul(pt[:],
                                     xv_tiles[(sb, dc)][:, j * P:(j + 1) * P],
                                     wv_sb[:, dc, :], start=False,
                                     stop=(dc == ND - 1))
                nc.vector.tensor_copy(
                    V[:, sc, :, 0:64],
                    pt[:].rearrange("p (h d) -> p h d", h=HG))
                yield 2300

        def out_sb(half, sb, yT_d):
            """Output projection columns sb for CT chunk pair half."""
            ss = slice(sb * 512, (sb + 1) * 512)
            for mc in range(ND):
                ms = slice(mc * P, (mc + 1) * P)
                pt = psum.tile([P, 512], F32, tag="proj", bufs=2,
                               name=f"py_{half}_{mc}_{sb}")
                for i, pcc in enumerate((2 * half, 2 * half + 1)):
                    nc.tensor.matmul(pt[:], wo_sb[:, pcc, ms],
                                     CT[:, pcc, ss], start=(i == 0),
                                     stop=(i == 1))
                st = spool.tile([P, 512], F16, tag="stage", bufs=3,
                                name=f"st_{half}_{mc}_{sb}")
                nc.vector.tensor_copy(st[:], pt[:])
                nc.sync.dma_start(yT_d[ms, ss], st[:])
                yield 600

        # ---------------- attention slot machinery ----------------
        at_tiles = {}

        def emit_scores(p, sq, k):
            qs = slice(sq * 512, (sq + 1) * 512)
            ks = slice(k * P, (k + 1) * P)
            sAB = psum.tile([P, 1024], F32, tag="sc", bufs=2,
                            name=f"sAB_{p}_{sq}_{k}")
            nc.tensor.matmul(sAB[:, 0:512], KT[0:64, p, ks],
                             QT[0:64, p, qs], start=True, stop=True)
            nc.tensor.matmul(sAB[:, 512:1024], KT[64:128, p, ks],
                             QT[64:128, p, qs], start=True, stop=True)
            eAB = epool.tile([P, 1024], F16, tag="exp", name=f"e_{p}_{sq}_{k}")
            nc.scalar.activation(eAB[:], sAB[:], Exp, bias=ebias[:])
            return eAB

        def emit_attn(p, sq, k, eAB):
            if k == 0:
                at_tiles[(p, sq)] = (
                    psum.tile([P, 512], F32, tag="atA", bufs=1,
                              name=f"atA_{p}_{sq}"),
                    psum.tile([P, 512], F32, tag="atB", bufs=1,
                              name=f"atB_{p}_{sq}"),
                )
            atA, atB = at_tiles[(p, sq)]
            nc.tensor.matmul(atA[0:65], V[:, k, 2 * p, 0:65], eAB[:, 0:512],
                             start=(k == 0), stop=(k == NSC - 1))
            nc.tensor.matmul(atB[0:65], V[:, k, 2 * p + 1, 0:65],
                             eAB[:, 512:1024], start=(k == 0),
                             stop=(k == NSC - 1))

        def evacuate(p, sq, g):
            """Raw attn -> CT (fp16); sums rows -> stage; defer normalize."""
            qs = slice(sq * 512, (sq + 1) * 512)
            atA, atB = at_tiles.pop((p, sq))
            stg = spool.tile([P, 2, 512], F32, tag="sums", bufs=2,
                             name=f"sums_{g}")
            nc.vector.tensor_copy(CT[0:64, p, qs], atA[0:64, :])
            nc.vector.tensor_copy(stg[64:65, 0, :], atA[64:65, :])
            tmpB = spool.tile([64, 512], F16, tag="tmpB", bufs=2,
                              name=f"tmpB_{g}")
            nc.vector.tensor_copy(tmpB[:], atB[0:64, :])
            nc.vector.tensor_copy(stg[64:65, 1, :], atB[64:65, :])
            nc.sync.dma_start(CT[64:128, p, qs], tmpB[:])
            return stg

        def norm_group(p, sq, g, stg):
            """Batched reciprocal + PE broadcast + one in-place multiply."""
            qs = slice(sq * 512, (sq + 1) * 512)
            rin = spool.tile([8, P], F32, tag="rin", bufs=2, name=f"rin_{g}")
            nc.sync.dma_start(rin[:], stg[64:65, :, :])
            rout = spool.tile([8, P], F32, tag="rout", bufs=2,
                              name=f"rout_{g}")
            with nc.allow_low_precision(
                    reason="softmax 1/sum: uniform per-column scale, well "
                           "within the output error budget"):
                nc.vector.reciprocal(rout[:], rin[:])
                r16 = spool.tile([8, P], F16, tag="r16", bufs=2,
                                 name=f"r16_{g}")
                nc.vector.tensor_copy(r16[:], rout[:])
            inv = spool.tile([1, 2, 512], F16, tag="inv", bufs=2,
                             name=f"inv_{g}")
            nc.sync.dma_start(inv[:], r16[:])
            yield 100
            bibc = psum.tile([P, 512], F32, tag="proj", bufs=2,
                             name=f"bibc_{g}")
            nc.tensor.matmul(bibc[0:64], ones[0:1, 0:64], inv[0:1, 0, :],
                             start=True, stop=True)
            nc.tensor.matmul(bibc[64:128], ones[0:1, 0:64], inv[0:1, 1, :],
                             start=True, stop=True)
            nc.vector.tensor_tensor(CT[:, p, qs], CT[:, p, qs], bibc[:], mult)
            yield 400

        # ---------------- top-level schedule ----------------
        def emit_all():
            filler = deque()       # credit-paced PE work
            norm_q = deque()       # priority steps (cheap, unblock output)
            credit = [0.0]
            SLOT_BUDGET = 650.0

            def step(q):
                gen = q[0]
                try:
                    cost = next(gen)
                    credit[0] -= cost
                    return True
                except StopIteration:
                    q.popleft()
                    return False

            def pace():
                credit[0] += SLOT_BUDGET
                if credit[0] > 4000.0 and not filler and not norm_q:
                    credit[0] = 4000.0
                while (norm_q or filler) and credit[0] > 0.0:
                    if norm_q:
                        step(norm_q)
                    else:
                        step(filler)

            # preamble: QK projection for pair 0 (dense; DMA-gated), with
            # V-projection chunks interleaved once the first x/w DMAs land.
            vgen = v_proj()
            v_left = [NSC]

            def step_v():
                if v_left[0] > 0:
                    try:
                        next(vgen)
                    except StopIteration:
                        pass
                    v_left[0] -= 1

            for i, _ in enumerate(qk_proj(0)):
                if i >= 2:
                    step_v()

            pend = deque()
            group_no = [0]

            def on_group_end(p, sq):
                g = group_no[0]
                group_no[0] += 1
                stg = evacuate(p, sq, g)
                ng = norm_group(p, sq, g, stg)
                if (p, sq) == (1, 3):
                    norm_q.append(_chain(ng, lambda: filler.append(
                        _cat(out_sb(0, 0, yTa), out_sb(0, 1, yTa),
                             out_sb(0, 2, yTa), out_sb(0, 3, yTa)))))
                elif p == 3:
                    norm_q.append(_chain(ng, lambda sq=sq: filler.append(
                        out_sb(1, sq, yTb))))
                else:
                    norm_q.append(ng)

            def pop_attn():
                it = pend.popleft()
                emit_attn(*it)
                if it[2] == NSC - 1:
                    on_group_end(it[0], it[1])

            for p in range(NCC):
                if p + 1 < NCC:
                    filler.append(qk_proj(p + 1))
                for sq in range(NSB):
                    for k in range(NSC):
                        eAB = emit_scores(p, sq, k)
                        pend.append((p, sq, k, eAB))
                        # group (0,0): keep stepping V; let pend grow so the
                        # ACT stream never waits on V availability.
                        if p == 0 and sq == 0:
                            step_v()
                            if len(pend) > 8:
                                pop_attn()
                        else:
                            if len(pend) > 6:
                                pop_attn()
                            if len(pend) > 6:
                                pop_attn()
                            # pull the k==15 pop 2 slots early so the psum
                            # evacuation clears before the next group's
                            # start=True matmuls reach the same banks
                            if pend and pend[0][2] >= 14 and len(pend) > 4:
                                pop_attn()
                        pace()
            while pend:
                pop_attn()
            while norm_q:
                step(norm_q)
            while filler:
                step(filler)

        def _chain(gen, then):
            yield from gen
            then()

        def _cat(*gens):
            for g in gens:
                yield from g

        emit_all()

    _split_waits(nc, max_waits=1)
    return nc


_PROGRAM = None


def _get_program():
    global _PROGRAM
    if _PROGRAM is None:
        _PROGRAM = build_program()
    return _PROGRAM


def _make_in_maps(q, k, v, Wq, bq, Wk, bk, Wv, bv, Wo, bo):
    f16 = np.float16
    xqT = [np.ascontiguousarray(q[b].T, dtype=f16) for b in range(B)]
    xkT = [np.ascontiguousarray(k[b].T, dtype=f16) for b in range(B)]
    xvT = [np.ascontiguousarray(v[b].T, dtype=f16) for b in range(B)]
    WqT = np.ascontiguousarray(Wq.T * 0.125, dtype=f16)
    WkT = np.ascontiguousarray(Wk.T, dtype=f16)
    WvT = np.ascontiguousarray(Wv.T, dtype=f16)
    WoT = np.ascontiguousarray(Wo.T, dtype=f16)
    in_maps = []
    for cid in range(N_CORES):
        b, hg = divmod(cid, 2)
        sl = slice(hg * C, (hg + 1) * C)
        in_maps.append({
            "xqT": xqT[b], "xkT": xkT[b], "xvT": xvT[b],
            "wqT": np.ascontiguousarray(WqT[:, sl]),
            "wkT": np.ascontiguousarray(WkT[:, sl]),
            "wvT": np.ascontiguousarray(WvT[:, sl]),
            "woT": np.ascontiguousarray(WoT[sl, :]),
            "bq": (bq[sl] * 0.125).astype(f16).reshape(1, C),
            "bk": bk[sl].astype(f16).reshape(1, C),
            "bv": bv[sl].astype(f16).reshape(1, C),
        })
    return in_maps


def run(inputs, trace=False, trace_cores=None):
    nc = _get_program()
    in_maps = _make_in_maps(**{k: np.asarray(v) for k, v in inputs.items()})
    res = bass_utils.run_bass_kernel_spmd(
        nc, in_maps, core_ids=list(range(N_CORES)), trace=trace,
        trace_cores=trace_cores)
    bo = np.asarray(inputs["bo"], dtype=np.float64)
    out = np.empty((B, S, D_MODEL), np.float32)
    for b in range(B):
        acc = (res.results[2 * b]["yTa"].astype(np.float64)
               + res.results[2 * b]["yTb"].astype(np.float64)
               + res.results[2 * b + 1]["yTa"].astype(np.float64)
               + res.results[2 * b + 1]["yTb"].astype(np.float64)).T + bo
        out[b] = acc.astype(np.float32)
    return out, res


def kernel(**inputs):
    out, _ = run(inputs, trace=False)
    return out


# revision 13
# speedup vs baseline: 1.4933x; 1.0913x over previous
"""Multi-head attention (B=4, S=2048, D=1024, H=16) on 8 TRN2 NeuronCores.

Sharding: core cid handles batch b = cid//2 and head-group hg = cid%2
(8 heads = 512 channels).  Each core computes, for its (b, hg):
  QT = (Wq_hg/8) @ q[b].T + bq/8      [512, 2048]  (channels on partitions)
  KT = Wk_hg @ k[b].T + bk            [512, 2048]
  V  = v[b] @ Wv_hg.T + bv            [2048, 512]  (seq on partitions)
  per head-pair p: scoresT chunk = KT_h^T-blocks @ QT_h (contraction d_k=64,
  two heads packed in the 128-partition dim via PE row tiles -> concurrent),
  softmax WITHOUT max-subtraction (exp bias -12 cancels in normalization),
  row-sums via a ones-column appended to V (65th matmul output row),
  attnT accumulated over S_k chunks in PSUM.
  Normalization is DEFERRED and BATCHED: raw attnT lands in CT (fp16);
  per (pair, sq-block) the two sum rows are staged, DMA-reshaped to
  [8,128], inverted with one reciprocal_approx_fast, DMA'd back to a
  single-partition row, PE-broadcast into a [128,512] psum tile via a
  col-tiled matmul pair, and applied with ONE tensor_tensor multiply.
  This keeps the PE instruction stream dense (no multi-us reciprocal on
  the critical path -> HAM stays at K=8/8).
  yT_partial halves (fp16) per head-pair-pair; host sums 4 partials/batch.

All PE operands are fp16; accumulation is fp32 in PSUM.
"""
import numpy as np
from collections import deque
from contextlib import ExitStack

import concourse.bass as bass
import concourse.tile as tile
import concourse.mybir as mybir
import concourse.bass_utils as bass_utils

D_MODEL = 1024
NHEAD = 16
D_K = 64
B = 4
S = 2048
N_CORES = 8
HG = 8            # heads per core
C = HG * D_K      # 512 channels per core
P = 128
EXP_BIAS = -12.0

F16 = mybir.dt.float16
F32 = mybir.dt.float32


def _split_waits(nc, max_waits=1):
    """Cayman CTRL/LW instruction structs carry a single sync-wait slot and
    this walrus rejects instructions with more; move excess SyncWaits onto
    injected same-engine NOPs placed immediately before the instruction."""
    n = 0
    for fn in nc.m.functions:
        for bb in fn.blocks:
            insts = list(bb.instructions)
            out = []
            changed = False
            for inst in insts:
                si = inst.sync_info
                waits = list(si.on_wait) if si is not None and si.on_wait else []
                if len(waits) > max_waits:
                    changed = True
                    extra, keep = waits[:-max_waits], waits[-max_waits:]
                    for w in extra:
                        n += 1
                        nop = mybir.InstNoOp(name=f"wsplit_{n}", ins=[], outs=[])
                        nop.engine = inst.engine
                        nop.sync_info = mybir.SyncInfo(on_wait=[w], on_update=[])
                        out.append(nop)
                    inst.sync_info = mybir.SyncInfo(
                        on_wait=keep,
                        on_update=list(si.on_update) if si.on_update else [],
                    )
                out.append(inst)
            if changed:
                bb.instructions = out
    return n


def build_program():
    NSB = S // 512     # 4 seq blocks
    NSC = S // P       # 16 seq chunks
    ND = D_MODEL // P  # 8 model-dim chunks
    NCC = C // P       # 4 head pairs

    nc = bass.Bass("TRN2", target_bir_lowering=False, debug=False,
                   num_devices=N_CORES)
    xq = nc.dram_tensor("xqT", [D_MODEL, S], F16, kind="ExternalInput").ap()
    xk = nc.dram_tensor("xkT", [D_MODEL, S], F16, kind="ExternalInput").ap()
    xv = nc.dram_tensor("xvT", [D_MODEL, S], F16, kind="ExternalInput").ap()
    wq = nc.dram_tensor("wqT", [D_MODEL, C], F16, kind="ExternalInput").ap()
    wk = nc.dram_tensor("wkT", [D_MODEL, C], F16, kind="ExternalInput").ap()
    wv = nc.dram_tensor("wvT", [D_MODEL, C], F16, kind="ExternalInput").ap()
    wo = nc.dram_tensor("woT", [C, D_MODEL], F16, kind="ExternalInput").ap()
    bq = nc.dram_tensor("bq", [1, C], F16, kind="ExternalInput").ap()
    bk = nc.dram_tensor("bk", [1, C], F16, kind="ExternalInput").ap()
    bv = nc.dram_tensor("bv", [1, C], F16, kind="ExternalInput").ap()
    yTa = nc.dram_tensor("yTa", [D_MODEL, S], F16, kind="ExternalOutput").ap()
    yTb = nc.dram_tensor("yTb", [D_MODEL, S], F16, kind="ExternalOutput").ap()

    Exp = mybir.ActivationFunctionType.Exp
    mult = mybir.AluOpType.mult

    with tile.TileContext(nc) as tc, ExitStack() as ctx:
        const = ctx.enter_context(tc.tile_pool(name="const", bufs=1))
        big = ctx.enter_context(tc.tile_pool(name="big", bufs=1))
        wpool = ctx.enter_context(tc.tile_pool(name="wp", bufs=1))
        xpool = ctx.enter_context(tc.tile_pool(name="xp", bufs=1))
        epool = ctx.enter_context(tc.tile_pool(name="ep", bufs=12))
        spool = ctx.enter_context(tc.tile_pool(name="sp", bufs=2))
        psum = ctx.enter_context(tc.tile_pool(name="ps", bufs=1, space="PSUM"))

        # ---- constants / weights; DMA order = priority order ----
        ones = const.tile([1, 512], F16, tag="ones")
        nc.vector.memset(ones[:], 1.0)
        ebias = const.tile([P, 1], F32, tag="ebias")
        nc.vector.memset(ebias[:], EXP_BIAS)
        dwarm = const.tile([P, 512], F16, tag="dwarm")
        nc.vector.memset(dwarm[:], 0.001)
        dexp = const.tile([P, 64], F16, tag="dexp")
        bq_sb = const.tile([1, C], F16, tag="bq")
        nc.sync.dma_start(bq_sb[:], bq)
        bk_sb = const.tile([1, C], F16, tag="bk")
        nc.sync.dma_start(bk_sb[:], bk)
        bv_sb = const.tile([1, C], F16, tag="bv")
        nc.sync.dma_start(bv_sb[:], bv)

        QT = big.tile([P, NCC, S], F16, tag="QT")
        KT = big.tile([P, NCC, S], F16, tag="KT")
        V = big.tile([P, NSC, HG, 66], F16, tag="V")
        CT = big.tile([P, NCC, S], F16, tag="CT")
        nc.vector.memset(V[:, :, :, 64:65], 1.0)

        wq_sb = wpool.tile([P, ND, C], F16, tag="wq", name="w_q")
        nc.sync.dma_start(wq_sb[:], wq.rearrange("(c p) m -> p c m", p=P))
        wk_sb = wpool.tile([P, ND, C], F16, tag="wk", name="w_k")
        nc.sync.dma_start(wk_sb[:], wk.rearrange("(c p) m -> p c m", p=P))

        # ACT table preload + PE HAM warm-up while the first DMAs stream.
        nc.scalar.activation(dexp[:], dwarm[:, 0:64], Exp, bias=ebias[:])
        for i in range(28):
            dpt = psum.tile([P, 512], F32, tag="proj", bufs=2, name=f"dw_{i}")
            nc.tensor.matmul(dpt[:], dwarm[:, 0:P], dwarm[:],
                             start=True, stop=True)

        # ---------------- projection generators ----------------
        def qk_proj(p):
            """QT/KT chunk for pair p; one yield per (tensor, s-block)."""
            for name, wd_sb, xd, b_sb, out_t in (
                    ("q", wq_sb, xq, bq_sb, QT), ("k", wk_sb, xk, bk_sb, KT)):
                xr = xd.rearrange("(c p) m -> p c m", p=P)
                for sb in range(NSB):
                    xt = xpool.tile([P, ND, 512], F16, tag="xt", bufs=3,
                                    name=f"x{name}{p}_{sb}")
                    nc.sync.dma_start(
                        xt[:], xr[:, :, sb * 512:(sb + 1) * 512])
                    pt = psum.tile([P, 512], F32, tag="proj", bufs=2,
                                   name=f"p{name}{p}_{sb}")
                    nc.tensor.matmul(pt[:], b_sb[0:1, p * P:(p + 1) * P],
                                     ones[0:1, :], start=True, stop=False)
                    for dc in range(ND):
                        nc.tensor.matmul(pt[:], wd_sb[:, dc, p * P:(p + 1) * P],
                                         xt[:, dc, :], start=False,
                                         stop=(dc == ND - 1))
                    nc.vector.tensor_copy(
                        out_t[:, p, sb * 512:(sb + 1) * 512], pt[:])
                    yield 2000

        wv_sb = wpool.tile([P, ND, C], F16, tag="wv", name="w_v")
        wo_sb = wpool.tile([P, NCC, D_MODEL], F16, tag="wo", name="w_o")
        xv_tiles = {}

        def v_proj():
            """V for ALL 8 heads per seq chunk (N=512 matmuls)."""
            nc.sync.dma_start(wv_sb[:], wv.rearrange("(c p) m -> p c m", p=P))
            nc.sync.dma_start(wo_sb[:], wo.rearrange("(c p) m -> p c m", p=P))
            xvr = xv.rearrange("(c p) m -> p c m", p=P)
            for sc in range(NSC):
                sb, j = divmod(sc, 4)
                if j == 0:
                    t = xpool.tile([P, ND, 512], F16, tag="xv", bufs=4,
                                   name=f"xv{sb}")
                    nc.sync.dma_start(
                        t[:], xvr[:, :, sb * 512:(sb + 1) * 512])
                    xv_tiles[sb] = t
                pt = psum.tile([P, 512], F32, tag="proj", bufs=2,
                               name=f"pv_{sc}")
                nc.tensor.matmul(pt[:], ones[0:1, 0:P], bv_sb[0:1, :],
                                 start=True, stop=False)
                for dc in range(ND):
                    nc.tensor.matmul(pt[:],
                                     xv_tiles[sb][:, dc, j * P:(j + 1) * P],
                                     wv_sb[:, dc, :], start=False,
                                     stop=(dc == ND - 1))
                nc.vector.tensor_copy(
                    V[:, sc, :, 0:64],
                    pt[:].rearrange("p (h d) -> p h d", h=HG))
                yield 2300

        def out_sb(half, sb, yT_d):
            """Output projection columns sb for CT chunk pair half."""
            ss = slice(sb * 512, (sb + 1) * 512)
            for mc in range(ND):
                ms = slice(mc * P, (mc + 1) * P)
                pt = psum.tile([P, 512], F32, tag="proj", bufs=2,
                               name=f"py_{half}_{mc}_{sb}")
                for i, pcc in enumerate((2 * half, 2 * half + 1)):
                    nc.tensor.matmul(pt[:], wo_sb[:, pcc, ms],
                                     CT[:, pcc, ss], start=(i == 0),
                                     stop=(i == 1))
                st = spool.tile([P, 512], F16, tag="stage", bufs=3,
                                name=f"st_{half}_{mc}_{sb}")
                nc.vector.tensor_copy(st[:], pt[:])
                nc.sync.dma_start(yT_d[ms, ss], st[:])
                yield 600

        # ---------------- attention slot machinery ----------------
        at_tiles = {}

        def emit_scores(p, sq, k):
            qs = slice(sq * 512, (sq + 1) * 512)
            ks = slice(k * P, (k + 1) * P)
            sAB = psum.tile([P, 1024], F32, tag="sc", bufs=2,
                            name=f"sAB_{p}_{sq}_{k}")
            nc.tensor.matmul(sAB[:, 0:512], KT[0:64, p, ks],
                             QT[0:64, p, qs], start=True, stop=True)
            nc.tensor.matmul(sAB[:, 512:1024], KT[64:128, p, ks],
                             QT[64:128, p, qs], start=True, stop=True)
            eAB = epool.tile([P, 1024], F16, tag="exp", name=f"e_{p}_{sq}_{k}")
            nc.scalar.activation(eAB[:], sAB[:], Exp, bias=ebias[:])
            return eAB

        def emit_attn(p, sq, k, eAB):
            if k == 0:
                at_tiles[(p, sq)] = (
                    psum.tile([P, 512], F32, tag="atA", bufs=1,
                              name=f"atA_{p}_{sq}"),
                    psum.tile([P, 512], F32, tag="atB", bufs=1,
                              name=f"atB_{p}_{sq}"),
                )
            atA, atB = at_tiles[(p, sq)]
            nc.tensor.matmul(atA[0:65], V[:, k, 2 * p, 0:65], eAB[:, 0:512],
                             start=(k == 0), stop=(k == NSC - 1))
            nc.tensor.matmul(atB[0:65], V[:, k, 2 * p + 1, 0:65],
                             eAB[:, 512:1024], start=(k == 0),
                             stop=(k == NSC - 1))

        def evacuate(p, sq, g):
            """Raw attn -> CT (fp16); sums rows -> stage; defer normalize."""
            qs = slice(sq * 512, (sq + 1) * 512)
            atA, atB = at_tiles.pop((p, sq))
            stg = spool.tile([P, 2, 512], F32, tag="sums", bufs=2,
                             name=f"sums_{g}")
            nc.vector.tensor_copy(CT[0:64, p, qs], atA[0:64, :])
            nc.vector.tensor_copy(stg[64:65, 0, :], atA[64:65, :])
            tmpB = spool.tile([64, 512], F16, tag="tmpB", bufs=2,
                              name=f"tmpB_{g}")
            nc.vector.tensor_copy(tmpB[:], atB[0:64, :])
            nc.vector.tensor_copy(stg[64:65, 1, :], atB[64:65, :])
            nc.sync.dma_start(CT[64:128, p, qs], tmpB[:])
            return stg

        def norm_group(p, sq, g, stg):
            """Batched reciprocal + PE broadcast + one in-place multiply."""
            qs = slice(sq * 512, (sq + 1) * 512)
            rin = spool.tile([8, P], F32, tag="rin", bufs=2, name=f"rin_{g}")
            nc.sync.dma_start(rin[:], stg[64:65, :, :])
            rout = spool.tile([8, P], F32, tag="rout", bufs=2,
                              name=f"rout_{g}")
            with nc.allow_low_precision(
                    reason="softmax 1/sum: uniform per-column scale, well "
                           "within the output error budget"):
                nc.vector.reciprocal(rout[:], rin[:])
                r16 = spool.tile([8, P], F16, tag="r16", bufs=2,
                                 name=f"r16_{g}")
                nc.vector.tensor_copy(r16[:], rout[:])
            inv = spool.tile([1, 2, 512], F16, tag="inv", bufs=2,
                             name=f"inv_{g}")
            nc.sync.dma_start(inv[:], r16[:])
            yield 100
            bibc = psum.tile([P, 512], F32, tag="proj", bufs=2,
                             name=f"bibc_{g}")
            nc.tensor.matmul(bibc[0:64], ones[0:1, 0:64], inv[0:1, 0, :],
                             start=True, stop=True)
            nc.tensor.matmul(bibc[64:128], ones[0:1, 0:64], inv[0:1, 1, :],
                             start=True, stop=True)
            nc.vector.tensor_tensor(CT[:, p, qs], CT[:, p, qs], bibc[:], mult)
            yield 400

        # ---------------- top-level schedule ----------------
        def emit_all():
            filler = deque()       # credit-paced PE work
            norm_q = deque()       # priority steps (cheap, unblock output)
            credit = [0.0]
            SLOT_BUDGET = 650.0

            def step(q):
                gen = q[0]
                try:
                    cost = next(gen)
                    credit[0] -= cost
                    return True
                except StopIteration:
                    q.popleft()
                    return False

            def pace():
                # cap so filler can never burst more than ~2 heavy yields
                # between consecutive attention slots (in-order PE queue:
                # a DMA-gated filler matmul would stall the scores behind it)
                credit[0] += SLOT_BUDGET
                if credit[0] > 4000.0 and not filler and not norm_q:
                    credit[0] = 4000.0
                while (norm_q or filler) and credit[0] > 0.0:
                    if norm_q:
                        step(norm_q)
                    else:
                        step(filler)

            # preamble: QK projection for pair 0 (dense; DMA-gated), with
            # V-projection chunks interleaved once the first x/w DMAs land.
            vgen = v_proj()
            v_left = [NSC]

            def step_v():
                if v_left[0] > 0:
                    try:
                        next(vgen)
                    except StopIteration:
                        pass
                    v_left[0] -= 1

            for i, _ in enumerate(qk_proj(0)):
                if i >= 2:
                    step_v()

            pend = deque()
            group_no = [0]

            def on_group_end(p, sq):
                g = group_no[0]
                group_no[0] += 1
                stg = evacuate(p, sq, g)
                ng = norm_group(p, sq, g, stg)
                if (p, sq) == (1, 3):
                    norm_q.append(_chain(ng, lambda: filler.append(
                        _cat(out_sb(0, 0, yTa), out_sb(0, 1, yTa),
                             out_sb(0, 2, yTa), out_sb(0, 3, yTa)))))
                elif p == 3:
                    norm_q.append(_chain(ng, lambda sq=sq: filler.append(
                        out_sb(1, sq, yTb))))
                else:
                    norm_q.append(ng)

            def pop_attn():
                it = pend.popleft()
                emit_attn(*it)
                if it[2] == NSC - 1:
                    on_group_end(it[0], it[1])

            for p in range(NCC):
                if p + 1 < NCC:
                    filler.append(qk_proj(p + 1))
                for sq in range(NSB):
                    for k in range(NSC):
                        eAB = emit_scores(p, sq, k)
                        pend.append((p, sq, k, eAB))
                        # group (0,0): keep stepping V; let pend grow so the
                        # ACT stream never waits on V availability.
                        if p == 0 and sq == 0:
                            step_v()
                            if len(pend) > 8:
                                pop_attn()
                        else:
                            if len(pend) > 6:
                                pop_attn()
                            if len(pend) > 6:
                                pop_attn()
                            # pull the k==15 pop 2 slots early so the psum
                            # evacuation clears before the next group's
                            # start=True matmuls reach the same banks
                            if pend and pend[0][2] >= 14 and len(pend) > 4:
                                pop_attn()
                        pace()
            while pend:
                pop_attn()
            while norm_q:
                step(norm_q)
            while filler:
                step(filler)

        def _chain(gen, then):
            yield from gen
            then()

        def _cat(*gens):
            for g in gens:
                yield from g

        emit_all()

    _split_waits(nc, max_waits=1)
    return nc


_PROGRAM = None


def _get_program():
    global _PROGRAM
    if _PROGRAM is None:
        _PROGRAM = build_program()
    return _PROGRAM


def _make_in_maps(q, k, v, Wq, bq, Wk, bk, Wv, bv, Wo, bo):
    f16 = np.float16
    xqT = [np.ascontiguousarray(q[b].T, dtype=f16) for b in range(B)]
    xkT = [np.ascontiguousarray(k[b].T, dtype=f16) for b in range(B)]
    xvT = [np.ascontiguousarray(v[b].T, dtype=f16) for b in range(B)]
    WqT = np.ascontiguousarray(Wq.T * 0.125, dtype=f16)
    WkT = np.ascontiguousarray(Wk.T, dtype=f16)
    WvT = np.ascontiguousarray(Wv.T, dtype=f16)
    WoT = np.ascontiguousarray(Wo.T, dtype=f16)
    in_maps = []
    for cid in range(N_CORES):
        b, hg = divmod(cid, 2)
        sl = slice(hg * C, (hg + 1) * C)
        in_maps.append({
            "xqT": xqT[b], "xkT": xkT[b], "xvT": xvT[b],
            "wqT": np.ascontiguousarray(WqT[:, sl]),
            "wkT": np.ascontiguousarray(WkT[:, sl]),
            "wvT": np.ascontiguousarray(WvT[:, sl]),
            "woT": np.ascontiguousarray(WoT[sl, :]),
            "bq": (bq[sl] * 0.125).astype(f16).reshape(1, C),
            "bk": bk[sl].astype(f16).reshape(1, C),
            "bv": bv[sl].astype(f16).reshape(1, C),
        })
    return in_maps


def run(inputs, trace=False, trace_cores=None):
    nc = _get_program()
    in_maps = _make_in_maps(**{k: np.asarray(v) for k, v in inputs.items()})
    res = bass_utils.run_bass_kernel_spmd(
        nc, in_maps, core_ids=list(range(N_CORES)), trace=trace,
        trace_cores=trace_cores)
    bo = np.asarray(inputs["bo"], dtype=np.float64)
    out = np.empty((B, S, D_MODEL), np.float32)
    for b in range(B):
        acc = (res.results[2 * b]["yTa"].astype(np.float64)
               + res.results[2 * b]["yTb"].astype(np.float64)
               + res.results[2 * b + 1]["yTa"].astype(np.float64)
               + res.results[2 * b + 1]["yTb"].astype(np.float64)).T + bo
        out[b] = acc.astype(np.float32)
    return out, res


def kernel(**inputs):
    out, _ = run(inputs, trace=False)
    return out


# revision 19
# speedup vs baseline: 1.5152x; 1.0147x over previous
"""Multi-head attention (B=4, S=2048, D=1024, H=16) on 8 TRN2 NeuronCores.

Sharding: core cid handles batch b = cid//2 and head-group hg = cid%2
(8 heads = 512 channels).  Each core computes, for its (b, hg):
  QT = (Wq_hg/8) @ q[b].T + bq/8      [512, 2048]  (channels on partitions)
  KT = Wk_hg @ k[b].T + bk            [512, 2048]
  V  = v[b] @ Wv_hg.T + bv            [2048, 512]  (seq on partitions)
  per head-pair p: scoresT chunk = KT_h^T-blocks @ QT_h (contraction d_k=64,
  two heads packed in the 128-partition dim via PE row tiles -> concurrent),
  softmax WITHOUT max-subtraction (exp bias -12 cancels in normalization),
  row-sums via a ones-column appended to V (65th matmul output row),
  attnT accumulated over S_k chunks in PSUM.
  Normalization is DEFERRED and BATCHED: raw attnT lands in CT (fp16);
  per (pair, sq-block) the two sum rows are staged, DMA-reshaped to
  [8,128], inverted with one reciprocal_approx_fast, DMA'd back to a
  single-partition row, PE-broadcast into a [128,512] psum tile via a
  col-tiled matmul pair, and applied with ONE tensor_tensor multiply.
  This keeps the PE instruction stream dense (no multi-us reciprocal on
  the critical path -> HAM stays at K=8/8).
  yT_partial halves (fp16) per head-pair-pair; host sums 4 partials/batch.

All PE operands are fp16; accumulation is fp32 in PSUM.
"""
import numpy as np
from collections import deque
from contextlib import ExitStack

import concourse.bass as bass
import concourse.tile as tile
import concourse.mybir as mybir
import concourse.bass_utils as bass_utils

D_MODEL = 1024
NHEAD = 16
D_K = 64
B = 4
S = 2048
N_CORES = 8
HG = 8            # heads per core
C = HG * D_K      # 512 channels per core
P = 128
EXP_BIAS = -12.0

F16 = mybir.dt.float16
F32 = mybir.dt.float32


def _split_waits(nc, max_waits=1):
    """Cayman CTRL/LW instruction structs carry a single sync-wait slot and
    this walrus rejects instructions with more; move excess SyncWaits onto
    injected same-engine NOPs placed immediately before the instruction."""
    n = 0
    for fn in nc.m.functions:
        for bb in fn.blocks:
            insts = list(bb.instructions)
            out = []
            changed = False
            for inst in insts:
                si = inst.sync_info
                waits = list(si.on_wait) if si is not None and si.on_wait else []
                if len(waits) > max_waits:
                    changed = True
                    extra, keep = waits[:-max_waits], waits[-max_waits:]
                    for w in extra:
                        n += 1
                        nop = mybir.InstNoOp(name=f"wsplit_{n}", ins=[], outs=[])
                        nop.engine = inst.engine
                        nop.sync_info = mybir.SyncInfo(on_wait=[w], on_update=[])
                        out.append(nop)
                    inst.sync_info = mybir.SyncInfo(
                        on_wait=keep,
                        on_update=list(si.on_update) if si.on_update else [],
                    )
                out.append(inst)
            if changed:
                bb.instructions = out
    return n


def build_program():
    NSB = S // 512     # 4 seq blocks
    NSC = S // P       # 16 seq chunks
    ND = D_MODEL // P  # 8 model-dim chunks
    NCC = C // P       # 4 head pairs

    nc = bass.Bass("TRN2", target_bir_lowering=False, debug=False,
                   num_devices=N_CORES)
    xq = nc.dram_tensor("xqT", [D_MODEL, S], F16, kind="ExternalInput").ap()
    xk = nc.dram_tensor("xkT", [D_MODEL, S], F16, kind="ExternalInput").ap()
    xv = nc.dram_tensor("xvT", [D_MODEL, S], F16, kind="ExternalInput").ap()
    wq = nc.dram_tensor("wqT", [D_MODEL, C], F16, kind="ExternalInput").ap()
    wk = nc.dram_tensor("wkT", [D_MODEL, C], F16, kind="ExternalInput").ap()
    wv = nc.dram_tensor("wvT", [D_MODEL, C], F16, kind="ExternalInput").ap()
    wo = nc.dram_tensor("woT", [C, D_MODEL], F16, kind="ExternalInput").ap()
    bq = nc.dram_tensor("bq", [1, C], F16, kind="ExternalInput").ap()
    bk = nc.dram_tensor("bk", [1, C], F16, kind="ExternalInput").ap()
    bv = nc.dram_tensor("bv", [1, C], F16, kind="ExternalInput").ap()
    yTa = nc.dram_tensor("yTa", [D_MODEL, S], F16, kind="ExternalOutput").ap()
    yTb = nc.dram_tensor("yTb", [D_MODEL, S], F16, kind="ExternalOutput").ap()

    Exp = mybir.ActivationFunctionType.Exp
    mult = mybir.AluOpType.mult

    with tile.TileContext(nc) as tc, ExitStack() as ctx:
        const = ctx.enter_context(tc.tile_pool(name="const", bufs=1))
        big = ctx.enter_context(tc.tile_pool(name="big", bufs=1))
        wpool = ctx.enter_context(tc.tile_pool(name="wp", bufs=1))
        xpool = ctx.enter_context(tc.tile_pool(name="xp", bufs=1))
        epool = ctx.enter_context(tc.tile_pool(name="ep", bufs=12))
        spool = ctx.enter_context(tc.tile_pool(name="sp", bufs=2))
        psum = ctx.enter_context(tc.tile_pool(name="ps", bufs=1, space="PSUM"))

        # ---- constants / weights; DMA order = priority order ----
        ones = const.tile([1, 512], F16, tag="ones")
        nc.vector.memset(ones[:], 1.0)
        ebias = const.tile([P, 1], F32, tag="ebias")
        nc.vector.memset(ebias[:], EXP_BIAS)
        dwarm = const.tile([P, 512], F16, tag="dwarm")
        nc.vector.memset(dwarm[:], 0.001)
        dexp = const.tile([P, 64], F16, tag="dexp")
        bq_sb = const.tile([1, C], F16, tag="bq")
        nc.sync.dma_start(bq_sb[:], bq)
        bk_sb = const.tile([1, C], F16, tag="bk")
        nc.sync.dma_start(bk_sb[:], bk)
        bv_sb = const.tile([1, C], F16, tag="bv")
        nc.sync.dma_start(bv_sb[:], bv)

        QT = big.tile([P, NCC, S], F16, tag="QT")
        KT = big.tile([P, NCC, S], F16, tag="KT")
        V = big.tile([P, NSC, HG, 66], F16, tag="V")
        CT = big.tile([P, NCC, S], F16, tag="CT")
        nc.vector.memset(V[:, :, :, 64:65], 1.0)

        wq_sb = wpool.tile([P, ND, C], F16, tag="wq", name="w_q")
        nc.sync.dma_start(wq_sb[:], wq.rearrange("(c p) m -> p c m", p=P))
        wk_sb = wpool.tile([P, ND, C], F16, tag="wk", name="w_k")
        nc.sync.dma_start(wk_sb[:], wk.rearrange("(c p) m -> p c m", p=P))

        # ACT table preload + PE HAM warm-up while the first DMAs stream.
        nc.scalar.activation(dexp[:], dwarm[:, 0:64], Exp, bias=ebias[:])
        for i in range(28):
            dpt = psum.tile([P, 512], F32, tag="proj", bufs=2, name=f"dw_{i}")
            nc.tensor.matmul(dpt[:], dwarm[:, 0:P], dwarm[:],
                             start=True, stop=True)

        # ---------------- projection generators ----------------
        def qk_proj(p, order=None):
            """QT/KT chunk for pair p; one yield per (tensor, s-block)."""
            parts = {"q": (wq_sb, xq, bq_sb, QT), "k": (wk_sb, xk, bk_sb, KT)}
            if order is None:
                order = [("q", sb) for sb in range(NSB)] + \
                        [("k", sb) for sb in range(NSB)]
            for name, sb in order:
                wd_sb, xd, b_sb, out_t = parts[name]
                xr = xd.rearrange("(c p) m -> p c m", p=P)
                if True:
                    xt = xpool.tile([P, ND, 512], F16, tag="xt", bufs=3,
                                    name=f"x{name}{p}_{sb}")
                    nc.sync.dma_start(
                        xt[:], xr[:, :, sb * 512:(sb + 1) * 512])
                    pt = psum.tile([P, 512], F32, tag="proj", bufs=2,
                                   name=f"p{name}{p}_{sb}")
                    nc.tensor.matmul(pt[:], b_sb[0:1, p * P:(p + 1) * P],
                                     ones[0:1, :], start=True, stop=False)
                    for dc in range(ND):
                        nc.tensor.matmul(pt[:], wd_sb[:, dc, p * P:(p + 1) * P],
                                         xt[:, dc, :], start=False,
                                         stop=(dc == ND - 1))
                    nc.vector.tensor_copy(
                        out_t[:, p, sb * 512:(sb + 1) * 512], pt[:])
                    yield 2000

        wv_sb = wpool.tile([P, ND, C], F16, tag="wv", name="w_v")
        wo_sb = wpool.tile([P, NCC, D_MODEL], F16, tag="wo", name="w_o")
        xv_tiles = {}

        def v_proj():
            """V for ALL 8 heads per seq chunk (N=512 matmuls)."""
            nc.sync.dma_start(wv_sb[:], wv.rearrange("(c p) m -> p c m", p=P))
            nc.sync.dma_start(wo_sb[:], wo.rearrange("(c p) m -> p c m", p=P))
            xvr = xv.rearrange("(c p) m -> p c m", p=P)
            for sc in range(NSC):
                sb, j = divmod(sc, 4)
                if j == 0:
                    t = xpool.tile([P, ND, 512], F16, tag="xv", bufs=4,
                                   name=f"xv{sb}")
                    nc.sync.dma_start(
                        t[:], xvr[:, :, sb * 512:(sb + 1) * 512])
                    xv_tiles[sb] = t
                pt = psum.tile([P, 512], F32, tag="proj", bufs=2,
                               name=f"pv_{sc}")
                nc.tensor.matmul(pt[:], ones[0:1, 0:P], bv_sb[0:1, :],
                                 start=True, stop=False)
                for dc in range(ND):
                    nc.tensor.matmul(pt[:],
                                     xv_tiles[sb][:, dc, j * P:(j + 1) * P],
                                     wv_sb[:, dc, :], start=False,
                                     stop=(dc == ND - 1))
                nc.vector.tensor_copy(
                    V[:, sc, :, 0:64],
                    pt[:].rearrange("p (h d) -> p h d", h=HG))
                yield 2300

        def out_sb(half, sb, yT_d):
            """Output projection columns sb for CT chunk pair half."""
            ss = slice(sb * 512, (sb + 1) * 512)
            for mc in range(ND):
                ms = slice(mc * P, (mc + 1) * P)
                pt = psum.tile([P, 512], F32, tag="proj", bufs=2,
                               name=f"py_{half}_{mc}_{sb}")
                for i, pcc in enumerate((2 * half, 2 * half + 1)):
                    nc.tensor.matmul(pt[:], wo_sb[:, pcc, ms],
                                     CT[:, pcc, ss], start=(i == 0),
                                     stop=(i == 1))
                st = spool.tile([P, 512], F16, tag="stage", bufs=3,
                                name=f"st_{half}_{mc}_{sb}")
                nc.vector.tensor_copy(st[:], pt[:])
                nc.sync.dma_start(yT_d[ms, ss], st[:])
                yield 600

        # ---------------- attention slot machinery ----------------
        at_tiles = {}

        def emit_scores(p, sq, k):
            qs = slice(sq * 512, (sq + 1) * 512)
            ks = slice(k * P, (k + 1) * P)
            sAB = psum.tile([P, 1024], F32, tag="sc", bufs=2,
                            name=f"sAB_{p}_{sq}_{k}")
            nc.tensor.matmul(sAB[:, 0:512], KT[0:64, p, ks],
                             QT[0:64, p, qs], start=True, stop=True)
            nc.tensor.matmul(sAB[:, 512:1024], KT[64:128, p, ks],
                             QT[64:128, p, qs], start=True, stop=True)
            eAB = epool.tile([P, 1024], F16, tag="exp", name=f"e_{p}_{sq}_{k}")
            nc.scalar.activation(eAB[:], sAB[:], Exp, bias=ebias[:])
            return eAB

        def emit_attn(p, sq, k, eAB):
            if k == 0:
                at_tiles[(p, sq)] = (
                    psum.tile([P, 512], F32, tag="atA", bufs=1,
                              name=f"atA_{p}_{sq}"),
                    psum.tile([P, 512], F32, tag="atB", bufs=1,
                              name=f"atB_{p}_{sq}"),
                )
            atA, atB = at_tiles[(p, sq)]
            nc.tensor.matmul(atA[0:65], V[:, k, 2 * p, 0:65], eAB[:, 0:512],
                             start=(k == 0), stop=(k == NSC - 1))
            nc.tensor.matmul(atB[0:65], V[:, k, 2 * p + 1, 0:65],
                             eAB[:, 512:1024], start=(k == 0),
                             stop=(k == NSC - 1))

        def evacuate(p, sq, g):
            """Raw attn -> CT (fp16); sums rows -> stage; defer normalize."""
            qs = slice(sq * 512, (sq + 1) * 512)
            atA, atB = at_tiles.pop((p, sq))
            stg = spool.tile([P, 2, 512], F32, tag="sums", bufs=2,
                             name=f"sums_{g}")
            nc.vector.tensor_copy(CT[0:64, p, qs], atA[0:64, :])
            nc.vector.tensor_copy(stg[64:65, 0, :], atA[64:65, :])
            tmpB = spool.tile([64, 512], F16, tag="tmpB", bufs=2,
                              name=f"tmpB_{g}")
            nc.vector.tensor_copy(tmpB[:], atB[0:64, :])
            nc.vector.tensor_copy(stg[64:65, 1, :], atB[64:65, :])
            nc.sync.dma_start(CT[64:128, p, qs], tmpB[:])
            return stg

        def norm_group(p, sq, g, stg):
            """Batched reciprocal + PE broadcast + one in-place multiply."""
            qs = slice(sq * 512, (sq + 1) * 512)
            rin = spool.tile([8, P], F32, tag="rin", bufs=2, name=f"rin_{g}")
            nc.sync.dma_start(rin[:], stg[64:65, :, :])
            rout = spool.tile([8, P], F32, tag="rout", bufs=2,
                              name=f"rout_{g}")
            with nc.allow_low_precision(
                    reason="softmax 1/sum: uniform per-column scale, well "
                           "within the output error budget"):
                nc.vector.reciprocal(rout[:], rin[:])
                r16 = spool.tile([8, P], F16, tag="r16", bufs=2,
                                 name=f"r16_{g}")
                nc.vector.tensor_copy(r16[:], rout[:])
            inv = spool.tile([1, 2, 512], F16, tag="inv", bufs=2,
                             name=f"inv_{g}")
            nc.sync.dma_start(inv[:], r16[:])
            yield 100
            bibc = psum.tile([P, 512], F32, tag="proj", bufs=2,
                             name=f"bibc_{g}")
            nc.tensor.matmul(bibc[0:64], ones[0:1, 0:64], inv[0:1, 0, :],
                             start=True, stop=True)
            nc.tensor.matmul(bibc[64:128], ones[0:1, 0:64], inv[0:1, 1, :],
                             start=True, stop=True)
            nc.vector.tensor_tensor(CT[:, p, qs], CT[:, p, qs], bibc[:], mult)
            yield 400

        # ---------------- top-level schedule ----------------
        def emit_all():
            filler = deque()       # credit-paced PE work
            norm_q = deque()       # priority steps (cheap, unblock output)
            credit = [0.0]
            SLOT_BUDGET = 650.0

            def step(q):
                gen = q[0]
                try:
                    cost = next(gen)
                    credit[0] -= cost
                    return True
                except StopIteration:
                    q.popleft()
                    return False

            def pace(budget=SLOT_BUDGET):
                credit[0] += budget
                if credit[0] > 4000.0 and not filler and not norm_q:
                    credit[0] = 4000.0
                while (norm_q or filler) and credit[0] > 0.0:
                    if norm_q:
                        step(norm_q)
                    else:
                        step(filler)

            # preamble: QK projection for pair 0 (dense; DMA-gated).  The
            # k-tensor blocks stream first (the k-sweep of group (0,0)
            # consumes ALL of KT within ~16 slots, while QT is needed one
            # s-block per group), and V chunks fill the PE while the large
            # x DMAs are still in flight.
            vgen = v_proj()
            v_left = [NSC]

            def step_v():
                if v_left[0] > 0:
                    try:
                        next(vgen)
                    except StopIteration:
                        pass
                    v_left[0] -= 1

            # dense preamble: just enough QK for the first slots; the rest
            # of pair 0's projection is the highest-priority filler so the
            # in-order PE queue reaches the first scores ASAP.
            for _ in qk_proj(0, [("q", 0), ("k", 0), ("k", 1)]):
                pass

            pend = deque()
            group_no = [0]

            def on_group_end(p, sq):
                g = group_no[0]
                group_no[0] += 1
                stg = evacuate(p, sq, g)
                ng = norm_group(p, sq, g, stg)
                if (p, sq) == (1, 3):
                    norm_q.append(_chain(ng, lambda: filler.append(
                        _cat(out_sb(0, 0, yTa), out_sb(0, 1, yTa),
                             out_sb(0, 2, yTa), out_sb(0, 3, yTa)))))
                elif p == 3:
                    norm_q.append(_chain(ng, lambda sq=sq: filler.append(
                        out_sb(1, sq, yTb))))
                else:
                    norm_q.append(ng)

            def pop_attn():
                it = pend.popleft()
                emit_attn(*it)
                if it[2] == NSC - 1:
                    on_group_end(it[0], it[1])

            for p in range(NCC):
                if p == 0:
                    filler.append(qk_proj(0, [("k", 2), ("k", 3), ("q", 1),
                                              ("q", 2), ("q", 3)]))
                if p + 1 < NCC:
                    filler.append(qk_proj(p + 1))
                for sq in range(NSB):
                    for k in range(NSC):
                        eAB = emit_scores(p, sq, k)
                        pend.append((p, sq, k, eAB))
                        # early groups: keep stepping V; let pend grow so
                        # the ACT stream never waits on V availability.
                        if p == 0 and sq == 0:
                            if k < 8 or k % 2 == 0:
                                step_v()
                            if len(pend) > 8:
                                pop_attn()
                        else:
                            if p == 0 and sq == 1:
                                step_v()
                            if len(pend) > 6:
                                pop_attn()
                            if len(pend) > 6:
                                pop_attn()
                            # pull the k==15 pop 2 slots early so the psum
                            # evacuation clears before the next group's
                            # start=True matmuls reach the same banks
                            if pend and pend[0][2] >= 14 and len(pend) > 4:
                                pop_attn()
                        pace(1000.0 if p == 3 else SLOT_BUDGET)
            while pend:
                pop_attn()
            while norm_q:
                step(norm_q)
            while filler:
                step(filler)

        def _chain(gen, then):
            yield from gen
            then()

        def _cat(*gens):
            for g in gens:
                yield from g

        emit_all()

    _split_waits(nc, max_waits=1)
    return nc


_PROGRAM = None


def _get_program():
    global _PROGRAM
    if _PROGRAM is None:
        _PROGRAM = build_program()
    return _PROGRAM


def _make_in_maps(q, k, v, Wq, bq, Wk, bk, Wv, bv, Wo, bo):
    f16 = np.float16
    xqT = [np.ascontiguousarray(q[b].T, dtype=f16) for b in range(B)]
    xkT = [np.ascontiguousarray(k[b].T, dtype=f16) for b in range(B)]
    xvT = [np.ascontiguousarray(v[b].T, dtype=f16) for b in range(B)]
    WqT = np.ascontiguousarray(Wq.T * 0.125, dtype=f16)
    WkT = np.ascontiguousarray(Wk.T, dtype=f16)
    WvT = np.ascontiguousarray(Wv.T, dtype=f16)
    WoT = np.ascontiguousarray(Wo.T, dtype=f16)
    in_maps = []
    for cid in range(N_CORES):
        b, hg = divmod(cid, 2)
        sl = slice(hg * C, (hg + 1) * C)
        in_maps.append({
            "xqT": xqT[b], "xkT": xkT[b], "xvT": xvT[b],
            "wqT": np.ascontiguousarray(WqT[:, sl]),
            "wkT": np.ascontiguousarray(WkT[:, sl]),
            "wvT": np.ascontiguousarray(WvT[:, sl]),
            "woT": np.ascontiguousarray(WoT[sl, :]),
            "bq": (bq[sl] * 0.125).astype(f16).reshape(1, C),
            "bk": bk[sl].astype(f16).reshape(1, C),
            "bv": bv[sl].astype(f16).reshape(1, C),
        })
    return in_maps


def run(inputs, trace=False, trace_cores=None):
    nc = _get_program()
    in_maps = _make_in_maps(**{k: np.asarray(v) for k, v in inputs.items()})
    res = bass_utils.run_bass_kernel_spmd(
        nc, in_maps, core_ids=list(range(N_CORES)), trace=trace,
        trace_cores=trace_cores)
    bo = np.asarray(inputs["bo"], dtype=np.float64)
    out = np.empty((B, S, D_MODEL), np.float32)
    for b in range(B):
        acc = (res.results[2 * b]["yTa"].astype(np.float64)
               + res.results[2 * b]["yTb"].astype(np.float64)
               + res.results[2 * b + 1]["yTa"].astype(np.float64)
               + res.results[2 * b + 1]["yTb"].astype(np.float64)).T + bo
        out[b] = acc.astype(np.float32)
    return out, res


def kernel(**inputs):
    out, _ = run(inputs, trace=False)
    return out
